# revision 52
# baseline (speedup 1.0000x reference)
"""GAT-style dense attention kernel for TRN2 (8 NeuronCores, SPMD over batch).

Reference computation (B=N=256, F=128, H=4, D=8):
  q = x@Wq+bq; k = x@Wk+bk; v = x@Wv+bv          (per-head dim D=8)
  s = einsum('bqhd,bkhd->bhqk', q, k)/sqrt(D)
  s = where(adj[q,k]==0, -inf, s)                 (adj shared across b,h)
  a = softmax(s, -1)
  out = einsum('bhqk,bkhd->bqhd', a, v).reshape(B,N,H*D) @ Wo + bo

Kernel strategy (per core: 32 batches):
  - host: xT = x.transpose -> [b, F, N] so contraction dim F is on partitions
  - qT/kT "spread" layout [128, N]: head h occupies partitions 32h..32h+8
    (one matmul each with host-prepared spread weights; scale 1/sqrt(D)
    folded into Wq; biases folded in on DVE during the PSUM->SBUF move)
  - scores S^T[k,q] per head-pair tile: mask addend written first by a
    single identity-matmul (stride-0 repeat over the pair), then K=8 score
    matmuls accumulate, 4 heads packed in PE row groups
  - exp on ScalarE straight out of PSUM -> bf16 E tiles (no max-subtraction:
    |s| <= ~8 for this distribution)
  - V and Wo fused on host: Wvo_h = Wv_h @ Wo_h; a ones column yields the
    softmax row-sums; per-head bias bv@Wo rides the ones trick (it divides
    out through the rowsum normalization)
  - P9 matmuls with E as STATIONARY ([128k x 128q] bf16 slices, Ldweights
    is free) and the 9-column V-block moving: output lands directly in the
    natural [q, (qchunk, h, 1+D)] layout -- no transpose needed
  - VectorE: reciprocal of rowsums, scale, sum over heads, +bo, DMA out
"""

import sys

sys.path.insert(0, "/opt/trn_rl_repo")

import numpy as np

import concourse.bass as bass
import concourse.tile as tile
from concourse import mybir
from concourse.bass import ts
from concourse.bass_utils import run_bass_kernel_spmd
from concourse.tile_rust import add_dep_helper


def _dep(from_inst, to_inst, reason):
    if from_inst is None or to_inst is None:
        return
    add_dep_helper(
        getattr(from_inst, "ins", from_inst),
        getattr(to_inst, "ins", to_inst),
        sync=False,
        reason=reason,
    )

DEBUG_LABELS = {}


def _lbl(inst, label):
    if inst is not None:
        m = getattr(inst, "ins", inst)
        DEBUG_LABELS[getattr(m, "name", "?")] = label
    return inst


B = 256
N = 256
F = 128
H = 4
D = 8
NCORES = 8
BPC = B // NCORES  # batches per core
MASK_NEG = -20.0
# Schraudolph bf16-exp constants: bits(bf16 e^s) ~= round(s * 2^7/ln2 + B16)
SCH_A16 = 184.6618
SCH_B16 = 16250.5
USE_SCHRAUDOLPH = False
WARM_REPS = 0

f32 = mybir.dt.float32
f32r = mybir.dt.float32r
bf16 = mybir.dt.bfloat16


def _build_consts(edge_index, Wq, bq, Wk, bk, Wv, bv, Wo, bo):
    scale = 1.0 / np.sqrt(np.float32(D))

    # spread projection weights: output partition 32h+d holds head h, dim d
    Wq_s = np.zeros((F, 128), np.float32)
    Wk_s = np.zeros((F, 128), np.float32)
    bqk = np.zeros((128, 2), np.float32)
    for h in range(H):
        for d in range(D):
            Wq_s[:, 32 * h + d] = Wq[:, 8 * h + d] * scale
            Wk_s[:, 32 * h + d] = Wk[:, 8 * h + d]
            bqk[32 * h + d, 0] = bq[8 * h + d] * scale
            bqk[32 * h + d, 1] = bk[8 * h + d]

    # fused V*Wo, 9 columns per head: col 9h+0 reserved (ones), 9h+1+j = VWo
    Wvo = np.zeros((F, 9 * H), np.float32)
    bvo = np.zeros((1, 9 * H), np.float32)
    for h in range(H):
        wv_h = Wv[:, 8 * h : 8 * h + 8]  # [F, 8]
        wo_h = Wo[8 * h : 8 * h + 8, :]  # [8, 8]
        Wvo[:, 9 * h + 1 : 9 * h + 9] = wv_h @ wo_h
        bvo[0, 9 * h + 1 : 9 * h + 9] = bv[8 * h : 8 * h + 8] @ wo_h
        bvo[0, 9 * h + 0] = 1.0  # ones column -> softmax row-sums
    # duplicated per k-chunk: vw move adds it as [128, (c 2, v 36)]
    bvo_full = np.broadcast_to(np.tile(bvo, (1, 2)), (128, 2 * 9 * H)).copy()

    # adjacency; mask addend M^T[k, q] packed as [128, 2, 256] (kchunk, q)
    adj = np.zeros((B, B), np.float32)
    adj[edge_index[0], edge_index[1]] = 1.0
    maskT = np.where(adj.T == 0.0, np.float32(MASK_NEG), np.float32(0.0))  # [k, q]
    maskT_p = np.ascontiguousarray(maskT.reshape(2, 128, 256).transpose(1, 0, 2))

    ident = np.eye(128, dtype=np.float32)
    bo_b = np.broadcast_to(bo.astype(np.float32), (128, D)).copy()


    # pack: cblob [128, 1014+576] = ident(128) | maskt(512 flat) | wqs(128) |
    # wks(128) | wvo(36) | bob(8) | bqk(2) | bvof(72) | ident2_f8(64) |
    # mask2_f8(512)
    cblob = np.concatenate(
        [
            ident,
            maskT_p.reshape(128, 512),
            Wq_s,
            Wk_s,
            Wvo,
            bo_b,
            bqk,
            bvo_full,
        ],
        axis=1,
    ).astype(np.float32)
    return dict(cblob=np.ascontiguousarray(cblob))


def _split_excess_waits(nc, max_waits=1):
    """Walrus allows only 2 sync-wait slots per engine instruction. Tile's
    vector-clock wait emission occasionally exceeds that (schedule-dependent);
    hoist the excess onto injected same-engine NoOps placed just before."""
    f = nc.m.functions[0]
    for bb in f.blocks:
        insts = list(bb.instructions)
        n_inserted = 0
        for idx, inst in enumerate(insts):
            si = getattr(inst, "sync_info", None)
            if si is None or not si.on_wait or len(si.on_wait) <= max_waits:
                continue
            waits = list(si.on_wait)
            keep, excess = waits[:max_waits], waits[max_waits:]
            pos = idx + n_inserted
            while excess:
                chunk, excess = excess[:max_waits], excess[max_waits:]
                nop = mybir.InstNoOp(
                    name=nc.get_next_instruction_name(),
                    ins=[],
                    outs=[],
                    engine=inst.engine,
                    sync_info=mybir.SyncInfo(on_wait=chunk, on_update=[]),
                    bass_nofuse=True,
                )
                bb.instructions.insert(pos, nop)
                pos += 1
                n_inserted += 1
            inst.sync_info = mybir.SyncInfo(on_wait=keep, on_update=si.on_update)


def _build_program():
    nc = bass.Bass()

    x_t = nc.declare_dram_parameter("xt", [BPC, F, N], f32r, isOutput=False)
    out = nc.declare_dram_parameter("out", [BPC, N, D], f32, isOutput=True)
    c_blob = nc.declare_dram_parameter("cblob", [128, 1014], f32r, isOutput=False)

    with tile.TileContext(nc) as tc:
        with (
            tc.tile_pool(name="consts", bufs=1) as cpool,
            tc.tile_pool(name="xt", bufs=33) as xt_pool,
            tc.tile_pool(name="qk", bufs=4) as qk_pool,
            tc.tile_pool(name="vw", bufs=3) as vw_pool,
            tc.tile_pool(name="E", bufs=8) as e_pool,
            tc.tile_pool(name="small", bufs=6) as sm_pool,
            tc.tile_pool(name="ostage", bufs=4) as ost_pool,
            tc.tile_pool(name="ps_qk", bufs=1, space="PSUM") as ps_qk_pool,
            tc.tile_pool(name="ps_vp", bufs=1, space="PSUM") as ps_vp_pool,
            tc.tile_pool(name="ps_s", bufs=3, space="PSUM") as ps_s,
        ):
            # first xt block rides the SP queue ahead of everything;
            # the const load follows on the ACT DGE queue
            xt0 = cpool.tile([128, 2, 128], f32r, tag="xt0")
            nc.sync.dma_start(
                out=xt0[:], in_=x_t[0].rearrange("f (c n) -> f c n", c=2)
            )
            cblob = cpool.tile([128, 1014], f32r, tag="cblob")
            nc.scalar.dma_start(out=cblob[:], in_=c_blob[:])

            identr = cblob[:, 0:128]
            masktf = cblob[:, 128:640]            # [128, 512] flat (c,q)
            wqs = cblob[:, 640:768]
            wks = cblob[:, 768:896]
            wvo = cblob[:, 896:932]
            bob = cblob[:, 932:940].bitcast(f32)
            bqk = cblob[:, 940:942].bitcast(f32)  # [128, 2] q/k bias
            bvof = cblob[:, 942:1014].bitcast(f32)  # [128, 72] vwo bias

            # PE p-state warmup: a tiny matmul as early as possible starts
            # the tensor-engine clock ramp before the real work arrives
            if WARM_REPS:
                warm_ps = ps_vp_pool.tile([128, 512], f32, tag="vp")
                warm_sb = cpool.tile([8, 512], f32r, tag="warm_sb")
                nc.vector.memset(warm_sb.bitcast(f32)[:], 0.0)
                for w in range(WARM_REPS):
                    nc.tensor.matmul(
                        warm_ps[0:8, :], warm_sb[:, 0:8], warm_sb[:],
                        start=True, stop=True, skip_group_check=(w > 0),
                    )

            # Make DVE and ACT observe the const-DMA queue once, so the
            # const-load ticks drop out of every later wait list (Tile's
            # vector-clock waits are not transitive across engines).
            obs = cpool.tile([1, 8], f32, tag="obs")
            nc.vector.tensor_copy(obs[:, 0:2], cblob[0:1, 0:2].bitcast(f32))
            nc.scalar.copy(obs[:, 4:6], cblob[0:1, 2:4].bitcast(f32))

            # per-batch state, filled by the stage emitters below
            xt_sb = {}      # b -> xt tile
            qk_ps = {}      # b -> PSUM qk tile
            vp_ps = {}      # b -> PSUM vw tile (vw [0:72])
            qk_sb = {}      # b -> SBUF qk tile
            vw_sb = {}      # b -> SBUF vw tile
            s_ps = {}       # (b, p) -> PSUM scores tile
            e_sb = {}       # (b, p) -> SBUF exp tile
            st = {"ostage": None, "tmp": None, "obsb": None,
                  "last_sc": None}

            def emit_xt(b):
                if not 0 <= b < BPC or b in xt_sb:
                    return
                t = xt_pool.tile([128, 2, 128], f32r, tag="xt")
                nc.sync.dma_start(
                    out=t[:], in_=x_t[b].rearrange("f (c n) -> f c n", c=2)
                )
                xt_sb[b] = t

            def emit_qkmm(b):
                # q^T/k^T spread projections into one PSUM bank
                if not 0 <= b < BPC:
                    return
                t = ps_qk_pool.tile([128, 2, 256], f32, tag="qkp")
                xt_flat = xt_sb[b].rearrange("f c n -> f (c n)")
                _lbl(nc.tensor.matmul(t[:, 0, :], wqs, xt_flat, start=True, stop=True), f"qmm({b})")
                _lbl(nc.tensor.matmul(
                    t[:, 1, :], wks, xt_flat,
                    start=True, stop=True, skip_group_check=True,
                ), f"kmm({b})")
                qk_ps[b] = t

            def emit_qkmove(b):
                # PSUM -> SBUF with per-partition q/k bias added in the move
                if not 0 <= b < BPC:
                    return
                t = qk_pool.tile([128, 2, 256], f32r, tag="qk")
                i = _lbl(nc.vector.tensor_add(
                    t[:], qk_ps[b][:],
                    bqk.unsqueeze(2).to_broadcast([128, 2, 256]),
                ), f"qkmove({b})")
                # the short pp->SBUF copy gates the next batch's scores (via
                # the shared vw+pp bank); keep this long move behind it on DVE
                _dep(i, st["tmp"], "ppcopy before qkmove on DVE")

                if st["obsb"] is None:
                    # absorb the xt DMA queue tick on DVE once
                    ob = sm_pool.tile([1, 2], f32, tag="obsb")
                    iob = nc.vector.tensor_copy(
                        ob[:], xt_sb[b][0:1, 0, 0:2].bitcast(f32)
                    )
                    _dep(i, iob, "absorb xt DMASW tick on DVE")
                    st["obsb"] = iob
                qk_sb[b] = t

            def emit_vwmm(b):
                # fused V*Wo projection into the shared vw+pp PSUM bank;
                # one tile serves a PAIR of batches (vw + pp for both fit in
                # one bank), so the tile-release chain binds only every
                # other batch, with two periods of slack
                if not 0 <= b < BPC:
                    return
                t = ps_vp_pool.tile([128, 512], f32, tag="vp")
                base = 0
                for c in range(2):
                    _lbl(nc.tensor.matmul(
                        t[:, base + 36 * c : base + 36 * c + 36],
                        xt_sb[b][:, c, :], wvo,
                        start=True, stop=True, skip_group_check=True,
                    ), f"vwmm({b},{c})")
                vp_ps[b] = t

            def emit_vwmove(b):
                if not 0 <= b < BPC:
                    return
                base = 0
                t = vw_pool.tile([128, 2, 9 * H], bf16, tag="vw")
                _lbl(nc.vector.tensor_add(
                    t[:],
                    vp_ps[b][:, base : base + 72].rearrange(
                        "p (c v) -> p c v", c=2
                    ),
                    bvof.rearrange("p (c v) -> p c v", c=2),
                ), f"vwmove({b})")
                vw_sb[b] = t

            def emit_mask_scores(b, p):
                if not 0 <= b < BPC:
                    return
                t = ps_s.tile([128, 2, 2, 256], f32, tag="S")  # (h%2, c, q)
                for hh_m in range(2):
                    _lbl(nc.tensor.matmul(
                        t[:, hh_m, :, :],
                        identr,
                        masktf,
                        start=True, stop=False,
                        skip_group_check=(hh_m == 1),
                    ), f"mask({b},{p},{hh_m})")
                for hh in range(2):
                    h = 2 * p + hh
                    for c in range(2):
                        i_sc = _lbl(nc.tensor.matmul(
                            t[:, hh, c, :],
                            qk_sb[b][32 * h : 32 * h + 8, 1, 128 * c : 128 * c + 128],
                            qk_sb[b][32 * h : 32 * h + 8, 0, :],
                            start=False, stop=(c == 1),
                            skip_group_check=True,
                            tile_position=(32 * h, 0),
                        ), f"sc({b},{p},h{h},c{c})")
                st["last_sc"] = i_sc
                s_ps[(b, p)] = t

            def emit_exp(b, p):
                if not 0 <= b < BPC:
                    return
                t = e_pool.tile([128, 2, 2, 256], bf16, tag="E")
                if p == 0 and USE_SCHRAUDOLPH:
                    # split the pair: head 0 exact exp on ACT, head 1 via
                    # the Schraudolph bit-trick on DVE --
                    # bf16(e^s) ~= bitcast_i16(round(s * 2^7/ln2 + B16))
                    _lbl(nc.scalar.activation(
                        t[:, 0], s_ps[(b, p)][:, 0],
                        mybir.ActivationFunctionType.Exp,
                    ), f"exp({b},{p})")
                    _lbl(nc.vector.tensor_scalar(
                        t.bitcast(mybir.dt.int16)[:, 1],
                        s_ps[(b, p)][:, 1],
                        SCH_A16, SCH_B16,
                        op0=mybir.AluOpType.mult, op1=mybir.AluOpType.add,
                    ), f"sexp({b},{p})")
                else:
                    _lbl(nc.scalar.activation(
                        t[:], s_ps[(b, p)][:], mybir.ActivationFunctionType.Exp
                    ), f"exp({b},{p})")
                e_sb[(b, p)] = t

            def emit_pp(b, p):
                # P9': E stationary, V-block moving -> natural [q, (c2,h,9)]
                # into this batch's pp region of the paired vw+pp bank
                if not 0 <= b < BPC:
                    return
                base = 256
                pp = vp_ps[b][:, base : base + 72].rearrange(
                    "p (c2 h j) -> p c2 h j", c2=2, h=H
                )
                e_p = e_sb[(b, p)]
                for hh in range(2):
                    h = 2 * p + hh
                    for c2 in range(2):
                        for c in range(2):
                            _lbl(nc.tensor.matmul(
                                pp[:, c2, h, :],
                                e_p[:, hh, c, 128 * c2 : 128 * c2 + 128],
                                vw_sb[b][:, c, 9 * h : 9 * h + 9],
                                start=(c == 0), stop=(c == 1),
                                skip_group_check=True,
                            ), f"pp({b},{p},h{h},c2{c2},c{c})")

            def emit_norm(b):
                if not 0 <= b < BPC:
                    return
                base = 256
                ppv = vp_ps[b][:, base : base + 72].rearrange(
                    "p (c2 h j) -> p c2 h j", c2=2, h=H
                )
                # tiny PSUM->SBUF copy is the only post-pp reader of the
                # paired bank; the divide runs on the idle Pool engine
                ppc = sm_pool.tile([128, 2, H, 9], f32, tag="ppc")
                i = _lbl(
                    nc.vector.tensor_copy(ppc[:], ppv[:]), f"ppcopy({b})"
                )
                st["tmp"] = i
                rec = sm_pool.tile([128, 2, H], f32, tag="rec")
                _lbl(nc.vector.reciprocal(rec[:], ppc[:, :, :, 0]), f"rec({b})")
                tmp = sm_pool.tile([128, 2, D, H], f32, tag="tmp")
                _lbl(nc.vector.tensor_mul(
                    tmp[:],
                    ppc[:, :, :, 1:9].transpose([0, 1, 3, 2]),
                    rec[:].unsqueeze(2).to_broadcast([128, 2, D, H]),
                ), f"mul({b})")
                st["div_tmp"] = tmp

            def emit_norm_b(b):
                if not 0 <= b < BPC:
                    return
                tmp = st["div_tmp"]
                red = sm_pool.tile([128, 2, D], f32, tag="red")
                _lbl(nc.vector.tensor_reduce(
                    red[:], tmp[:], axis=mybir.AxisListType.X,
                    op=mybir.AluOpType.add,
                ), f"red({b})")
                if b % 8 == 0:
                    ostage = ost_pool.tile([128, 8, 2, D], f32, tag="ost")
                    st["ostage"] = ostage
                nc.vector.tensor_add(
                    st["ostage"][:, b % 8, :, :],
                    red[:],
                    bob.unsqueeze(1).to_broadcast([128, 2, D]),
                )
                if b % 8 == 7:
                    nc.sync.dma_start(
                        out=out[b - 7 : b + 1].rearrange(
                            "b (c p) j -> p b c j", c=2
                        ),
                        in_=st["ostage"][:],
                    )

            def drop(b):
                # release python refs so tile pools can recycle cleanly
                for d in (xt_sb, qk_ps, vp_ps, qk_sb, vw_sb):
                    d.pop(b, None)
                for p in range(2):
                    s_ps.pop((b, p), None)
                    e_sb.pop((b, p), None)

            # ---- software-pipelined schedule ----
            # prologue
            xt_sb[0] = xt0
            emit_xt(0)
            emit_xt(1)
            emit_xt(2)
            emit_qkmm(0)
            emit_qkmove(0)
            emit_vwmm(0)
            emit_vwmove(0)
            emit_mask_scores(0, 0)
            emit_exp(0, 0)
            emit_qkmm(1)
            emit_qkmove(1)
            emit_mask_scores(0, 1)
            emit_qkmm(2)
            emit_qkmove(2)
            emit_mask_scores(1, 0)

            # scores are emitted two pairs ahead of the exp-gated pp groups,
            # so PE blocking at pp never delays the next exps' inputs
            for b in range(BPC):
                emit_xt(b + 3)
                emit_exp(b, 1)
                emit_mask_scores(b + 1, 1)
                emit_exp(b + 1, 0)
                emit_mask_scores(b + 2, 0)
                emit_pp(b, 0)
                emit_qkmm(b + 3)
                emit_pp(b, 1)
                emit_norm(b)
                emit_vwmm(b + 1)
                emit_vwmove(b + 1)
                emit_qkmove(b + 3)
                emit_norm_b(b)
                drop(b)

    _split_excess_waits(nc)
    return nc


_NC_CACHE = None
LAST_RESULTS = None


def kernel(**inputs) -> np.ndarray:
    global _NC_CACHE
    x = np.asarray(inputs["x"], np.float32)
    edge_index = np.asarray(inputs["edge_index"])
    consts = _build_consts(
        edge_index,
        np.asarray(inputs["Wq"], np.float32), np.asarray(inputs["bq"], np.float32),
        np.asarray(inputs["Wk"], np.float32), np.asarray(inputs["bk"], np.float32),
        np.asarray(inputs["Wv"], np.float32), np.asarray(inputs["bv"], np.float32),
        np.asarray(inputs["Wo"], np.float32), np.asarray(inputs["bo"], np.float32),
    )

    if _NC_CACHE is None:
        _NC_CACHE = _build_program()
    nc = _NC_CACHE

    in_maps = []
    for core in range(NCORES):
        xs = x[core * BPC : (core + 1) * BPC]  # [BPC, N, F]
        xt = np.ascontiguousarray(xs.transpose(0, 2, 1))  # [BPC, F, N]
        m = {"xt": xt}
        m.update(consts)
        in_maps.append(m)

    res = run_bass_kernel_spmd(nc, in_maps, list(range(NCORES)))
    global LAST_RESULTS
    LAST_RESULTS = res
    outs = [res.results[i]["out"] for i in range(NCORES)]
    return np.concatenate(outs, axis=0).astype(np.float32)


if __name__ == "__main__":
    rng = np.random.default_rng(0)
    demo = dict(
        x=rng.standard_normal((B, N, F), dtype=np.float32),
        edge_index=np.concatenate(
            [rng.integers(0, B, (2, 8192)), np.stack([np.arange(B)] * 2)], axis=1
        ).astype(np.int32),
        Wq=rng.standard_normal((F, H * D), dtype=np.float32) / np.sqrt(F),
        bq=rng.standard_normal(H * D, dtype=np.float32) / np.sqrt(F),
        Wk=rng.standard_normal((F, H * D), dtype=np.float32) / np.sqrt(F),
        bk=rng.standard_normal(H * D, dtype=np.float32) / np.sqrt(F),
        Wv=rng.standard_normal((F, H * D), dtype=np.float32) / np.sqrt(F),
        bv=rng.standard_normal(H * D, dtype=np.float32) / np.sqrt(F),
        Wo=rng.standard_normal((H * D, D), dtype=np.float32) / np.sqrt(H * D),
        bo=rng.standard_normal(D, dtype=np.float32) / np.sqrt(H * D),
    )
    out = kernel(**demo)
    print("kernel output", out.shape, out.dtype)
